# revision 29
# baseline (speedup 1.0000x reference)
"""GNN message-passing layer (GSS GNNLayer) on 8 Trainium2 NeuronCores.

Math (see reference):
    Ax   = A @ x                 (sparse COO, E edges)
    pre1 = Ax @ W1.T + b1
    Axx  = A @ (Ax * x)
    pre2 = Axx @ W2.T + b2
    pre  = pre1 + pre2 ; out = elu(pre) ; return (pre, out)

Distribution: row-partition by destination node; core c owns dest rows
[c*5000, (c+1)*5000). Edges are bucketed by (core, dest-block of 128,
lo/hi source-table half) on the host, sorted by source within a bucket
(HBM locality), and padded to chunks of 128 with val=0 edges.

All data-plane tensors are bf16 (2e-2 rel-err budget; bf16 lands ~2e-3).
Pass 2 exploits Ax + Axx = A @ (x + Ax*x): it gathers H' = x*(1+Ax) and
accumulates (Ax+Axx) directly, so with W1 == W2 (always true for this
module's init) a single dense matmul per block finishes the layer:
    pre = (Ax+Axx) @ W1.T + (b1+b2)
(W1 != W2 falls back to an extra Ax @ (W1-W2).T matmul per block.)

SpMM: per chunk of 128 edges the device dma_gather's the 128 source
rows (256 B/row bf16, int16 indices, table halved at row 32768, 4
SWDGE queues round-robin), builds one bf16 selection matrix
    S[e,d] = val[e] * (d == rowlocal[e])
with a fused tensor_scalar (alternating DVE / GpSimd to split the
sequencer load), and accumulates S.T @ M (pass 1) or M.T @ S (pass 2,
transposed result feeds the dense matmul without a PE transpose) into
the block's fp32 PSUM tile.

Finals: bias via a rank-1 matmul (ones.T @ bsum_row) into the same
PSUM, ELU = max(x, min(exp(x),1)-1) split across Activation + DVE.

SPMD: one program for all 8 cores; per-(block,table) chunk counts are
the max over cores, computed from the actual input, so the program
structure is uniform and only the data differs.
"""

import os
import numpy as np
import ml_dtypes

BF16 = ml_dtypes.bfloat16

N = 40000
D = 128
E = 640000
NCORES = 8
NSH = N // NCORES          # 5000 dest rows per core
P = 128
NB = (NSH + P - 1) // P    # 40 dest blocks per core (last has 8 rows)
SPLIT = 32768              # int16 gather index limit
NQ = 4                     # SWDGE queues for gathers

# tuning knobs (baked into the compiled program; cache-keyed)
SBW = int(os.environ.get("K_SBW", "2"))      # blocks per gather super-block
TSDVE = int(os.environ.get("K_TSDVE", "3"))  # S-builds on DVE out of 3
ACTMIN = os.environ.get("K_ACTMIN", "1") == "1"  # Act engine: Exp only
ODMA = os.environ.get("K_ODMA", "sync")      # output-DMA engine
SPKT = os.environ.get("K_SPKT", "0") == "1"  # dma_gather single_packet
MBUF = int(os.environ.get("K_MBUF", "3"))    # gather-tile ring depth
SPOOL = int(os.environ.get("K_SPOOL", "12"))  # S-tile ring depth
PSEG = int(os.environ.get("K_PSEG", "4"))    # segment-PSUM ring depth
DEFER = int(os.environ.get("K_DEFER", "0"))  # finalize deferral (superblocks)
HOSTS = os.environ.get("K_HOSTS", "1") == "1"  # host-built S tiles via DMA

_cache = {}


def _preprocess(adj_row, adj_col, adj_val):
    """Bucket/pad edges; build per-core gather-index and S-descriptor arrays."""
    row = np.asarray(adj_row, np.int64)
    col = np.asarray(adj_col, np.int64)
    val = np.asarray(adj_val, np.float32)

    core = row // NSH
    loc = row - core * NSH
    blk = loc // P
    dloc = (loc % P).astype(np.float32)
    hi = (col >= SPLIT).astype(np.int64)

    key = (core * NB + blk) * 2 + hi          # 0 .. NCORES*NB*2-1
    nkey = NCORES * NB * 2
    order = np.lexsort((col, key))            # bucket-major, source-sorted
    sk = key[order]
    counts = np.bincount(key, minlength=nkey)
    gstart = np.concatenate([[0], np.cumsum(counts)[:-1]])
    pos = np.arange(len(sk)) - gstart[sk]     # rank within its bucket
    cnt = counts.reshape(NCORES, NB, 2)
    caps = np.ceil(cnt / P).astype(np.int64).max(axis=0)   # [NB, 2]
    caps[:, 0] = np.maximum(caps[:, 0], 1)    # every block needs >=1 chunk
    caps_lo = caps[:, 0]
    caps_hi = caps[:, 1]

    # chunk-column layout: per block, lo chunks then hi chunks
    col0 = np.zeros((NB, 2), np.int64)
    run = 0
    for b in range(NB):
        col0[b, 0] = run
        run += caps_lo[b]
        col0[b, 1] = run
        run += caps_hi[b]
    TC = int(run)
    lostart = np.concatenate([[0], np.cumsum(caps_lo)])
    histart = np.concatenate([[0], np.cumsum(caps_hi)])
    CL = int(lostart[-1]) * 8                 # idx cols (16 idx/col)
    CH = max(int(histart[-1]) * 8, 1)

    rowloc = np.zeros((NCORES, P, TC), np.float32)
    vhi = np.zeros((NCORES, P, TC), np.float32)
    idxlo = np.zeros((NCORES, P, CL), np.int16)
    idxhi = np.zeros((NCORES, P, CH), np.int16)
    sall = np.zeros((NCORES, P, TC, P), BF16)   # host-built S tiles

    cS = sk // (NB * 2)
    bS = (sk // 2) % NB
    tS = sk % 2
    dS = dloc[order]
    vS = val[order]
    colS = col[order]

    ccol = col0[bS, tS] + pos // P
    pp = pos % P
    rowloc[cS, pp, ccol] = dS
    vhi[cS, pp, ccol] = vS
    sall[cS, pp, ccol, dS.astype(np.int64)] = vS.astype(BF16)

    reps = 16 * np.arange(8)[None, :]
    m = tS == 0
    q = lostart[bS[m]] * P + pos[m]
    idxlo[cS[m][:, None], (q % 16)[:, None] + reps, (q // 16)[:, None]] = \
        colS[m].astype(np.int16)[:, None]
    m = tS == 1
    if m.any():
        q = histart[bS[m]] * P + pos[m]
        idxhi[cS[m][:, None], (q % 16)[:, None] + reps, (q // 16)[:, None]] = \
            (colS[m] - SPLIT).astype(np.int16)[:, None]

    return dict(caps_lo=tuple(int(x) for x in caps_lo),
                caps_hi=tuple(int(x) for x in caps_hi),
                TC=TC, CL=CL, CH=CH,
                rowloc=rowloc, vhi=vhi,
                idxlo=idxlo, idxhi=idxhi,
                sall=sall.reshape(NCORES, P, TC * P))


def _build(caps_lo, caps_hi, TC, CL, CH, reps=1, fused=True):
    ABL = set(os.environ.get('ABL', '').split(','))
    import concourse.bacc as bacc
    import concourse.mybir as mybir
    import concourse.tile as tile

    f32 = mybir.dt.float32
    bf16 = mybir.dt.bfloat16
    i16 = mybir.dt.int16
    Alu = mybir.AluOpType
    Act = mybir.ActivationFunctionType

    lostart = np.concatenate([[0], np.cumsum(caps_lo)]).astype(int)
    histart = np.concatenate([[0], np.cumsum(caps_hi)]).astype(int)
    col0 = np.zeros((NB, 2), np.int64)
    run = 0
    for b in range(NB):
        col0[b, 0] = run
        run += caps_lo[b]
        col0[b, 1] = run
        run += caps_hi[b]

    nc = bacc.Bacc(None, target_bir_lowering=False, num_swdge_queues=NQ)
    xtab = nc.declare_dram_parameter("xtab", [N, D], bf16, isOutput=False)
    xsh = nc.declare_dram_parameter("xshard", [NSH, D], f32, isOutput=False)
    idxlo_d = nc.declare_dram_parameter("idxlo", [P, CL], i16, isOutput=False)
    idxhi_d = nc.declare_dram_parameter("idxhi", [P, CH], i16, isOutput=False)
    if HOSTS:
        sall_d = nc.declare_dram_parameter("sall", [P, TC * P], bf16,
                                           isOutput=False)
    else:
        rowloc_d = nc.declare_dram_parameter("rowloc", [P, TC], f32,
                                             isOutput=False)
        vhi_d = nc.declare_dram_parameter("vhi", [P, TC], f32, isOutput=False)
    wsum_d = nc.declare_dram_parameter("wsum", [D, D], f32, isOutput=False)
    bsum_d = nc.declare_dram_parameter("bsum", [1, D], f32, isOutput=False)
    if not fused:
        wdiff_d = nc.declare_dram_parameter("wdiff", [D, D], f32, isOutput=False)
    pre_o = nc.declare_dram_parameter("pre", [NSH, D], f32, isOutput=True)
    elu_o = nc.declare_dram_parameter("eluout", [NSH, D], f32, isOutput=True)
    h2sh = nc.dram_tensor("H2_shard", [NSH, D], bf16)
    h2full = nc.dram_tensor("H2_full", [N, D], bf16, addr_space="Shared")

    NSB = (NB + SBW - 1) // SBW
    nlo_sb = [sum(caps_lo[s * SBW:(s + 1) * SBW]) for s in range(NSB)]
    nhi_sb = [sum(caps_hi[s * SBW:(s + 1) * SBW]) for s in range(NSB)]
    NROT = 8

    with tile.TileContext(nc) as tc:
        with (
            tc.tile_pool(name="const", bufs=1) as cpool,
            tc.tile_pool(name="mlo", bufs=MBUF) as mlop,
            tc.tile_pool(name="mhi", bufs=MBUF) as mhip,
            tc.tile_pool(name="sel", bufs=SPOOL) as spool,
            tc.tile_pool(name="ssb", bufs=MBUF) as sspool,
            tc.tile_pool(name="small", bufs=3) as smp,
            tc.tile_pool(name="psum", bufs=PSEG, space="PSUM") as pseg,
            tc.tile_pool(name="psum3", bufs=2, space="PSUM") as ppre,
        ):
            if not HOSTS:
                iota_v = cpool.tile([P, P], bf16)
                nc.gpsimd.iota(iota_v[:], pattern=[[1, P]], base=0,
                               channel_multiplier=0,
                               allow_small_or_imprecise_dtypes=True)
                iota_p = cpool.tile([P, P], bf16)
                nc.vector.tensor_copy(iota_p[:], iota_v[:])
            wsum_t = cpool.tile([D, D], f32)
            nc.sync.dma_start(wsum_t[:], wsum_d[:])
            bsum_t = cpool.tile([1, D], f32)
            nc.sync.dma_start(bsum_t[:], bsum_d[:])
            idxlo_t = cpool.tile([P, CL], i16)
            nc.sync.dma_start(idxlo_t[:], idxlo_d[:])
            idxhi_t = cpool.tile([P, CH], i16)
            nc.sync.dma_start(idxhi_t[:], idxhi_d[:])
            if not HOSTS:
                rowloc_t = cpool.tile([P, TC], f32)
                nc.sync.dma_start(rowloc_t[:], rowloc_d[:])
                vhi_t = cpool.tile([P, TC], f32)
                nc.sync.dma_start(vhi_t[:], vhi_d[:])
            # full x shard resident in SBUF: [dest_p, block, D]
            xall = cpool.tile([P, NB, D], f32)
            for b in range(NB):
                rows = min(P, NSH - b * P)
                nc.sync.dma_start(xall[:rows, b, :],
                                  xsh[b * P:b * P + rows, :])
            # rotating operand copies for the per-block dense finals
            wr, onesr, bsr = [], [], []
            for k in range(NROT):
                t1 = cpool.tile([D, D], bf16, tag=f"wr{k}")
                nc.vector.tensor_copy(t1[:], wsum_t[:])
                wr.append(t1)
                t2 = cpool.tile([1, P], bf16, tag=f"ones{k}")
                nc.vector.memset(t2[:], 1.0)
                onesr.append(t2)
                t3 = cpool.tile([1, D], bf16, tag=f"bsr{k}")
                nc.vector.tensor_copy(t3[:], bsum_t[:])
                bsr.append(t3)
            if not fused:
                from concourse.masks import make_identity
                ident = cpool.tile([P, P], f32)
                make_identity(nc, ident[:])
                wdiff_t = cpool.tile([D, D], f32)
                nc.sync.dma_start(wdiff_t[:], wdiff_d[:])
                wdr, idr = [], []
                for k in range(NROT):
                    t4 = cpool.tile([D, D], bf16, tag=f"wd{k}")
                    nc.vector.tensor_copy(t4[:], wdiff_t[:])
                    wdr.append(t4)
                    t5 = cpool.tile([P, P], f32, tag=f"idr{k}")
                    nc.vector.tensor_copy(t5[:], ident[:])
                    idr.append(t5)
                ax_all = cpool.tile([P, NB * P], f32)

            odma = nc.sync if ODMA == "sync" else nc.scalar
            qctr = [0]
            sctr = [0]

            def run_once():
                def spmm_pass(tlo, thi, finalize, swapT):
                    # Deferring finalize emission by DEFER superblocks keeps
                    # the next superblock's S-builds ahead of fin ops in the
                    # in-order engine queues (software pipelining).
                    pending = []

                    def flush(limit):
                        while len(pending) > limit:
                            fb, fps = pending.pop(0)
                            finalize(fb, fps)

                    for s in range(NSB):
                        b0 = s * SBW
                        nlo, nhi = nlo_sb[s], nhi_sb[s]
                        if HOSTS:
                            c0 = int(col0[b0, 0])
                            nsb = nlo + nhi
                            s_sb = sspool.tile([P, nsb, P], bf16, tag="ssb")
                            nc.sync.dma_start(
                                s_sb[:],
                                sall_d[:, c0 * P:(c0 + nsb) * P])
                        mlo = mlop.tile([P, nlo, D], bf16, tag="mlo")
                        if 'nogather' in ABL:
                            nc.sync.dma_start(mlo[:, 0, :], tlo[0:P, :])
                        else:
                            h1 = nlo // 2
                            for (g0, g1) in ((0, h1), (h1, nlo)):
                                if g1 <= g0:
                                    continue
                                nc.gpsimd.dma_gather(
                                    out_ap=mlo[:, g0:g1, :], in_ap=tlo,
                                    idxs_ap=idxlo_t[:, (lostart[b0] + g0) * 8:(lostart[b0] + g1) * 8],
                                    num_idxs=(g1 - g0) * P,
                                    num_idxs_reg=(g1 - g0) * P,
                                    elem_size=D, single_packet=SPKT,
                                    queue_num=qctr[0] % NQ)
                                qctr[0] += 1
                        if nhi:
                            mhi = mhip.tile([P, nhi, D], bf16, tag="mhi")
                            if 'nogather' in ABL:
                                nc.sync.dma_start(mhi[:, 0, :], tlo[0:P, :])
                            else:
                                nc.gpsimd.dma_gather(
                                    out_ap=mhi[:], in_ap=thi,
                                    idxs_ap=idxhi_t[:, histart[b0] * 8:(histart[b0] + nhi) * 8],
                                    num_idxs=nhi * P, num_idxs_reg=nhi * P,
                                    elem_size=D, single_packet=SPKT,
                                    queue_num=qctr[0] % NQ)
                                qctr[0] += 1
                        for i in range(min(SBW, NB - b0)):
                            b = b0 + i
                            ps = pseg.tile([P, P], f32, tag="seg")
                            tot = caps_lo[b] + caps_hi[b]
                            done = 0
                            glo = sum(caps_lo[b0:b])
                            ghi = sum(caps_hi[b0:b])
                            for j in range(tot):
                                if j < caps_lo[b]:
                                    c = int(col0[b, 0]) + j
                                    msl = mlo[:, glo + j, :]
                                else:
                                    c = int(col0[b, 1]) + (j - caps_lo[b])
                                    msl = mhi[:, ghi + (j - caps_lo[b]), :]
                                if 'noseg' in ABL:
                                    done += 1
                                    continue
                                if HOSTS:
                                    sv = s_sb[:, c - c0, :]
                                else:
                                    svh = spool.tile([P, P], bf16, tag="S")
                                    # DVE : GpSimd split of the S-builds
                                    if sctr[0] % 3 < TSDVE:
                                        nc.vector.tensor_scalar(
                                            svh[:], iota_v[:],
                                            rowloc_t[:, c:c + 1],
                                            vhi_t[:, c:c + 1],
                                            op0=Alu.is_equal, op1=Alu.mult)
                                    else:
                                        nc.gpsimd.tensor_scalar(
                                            svh[:], iota_p[:],
                                            rowloc_t[:, c:c + 1],
                                            vhi_t[:, c:c + 1],
                                            op0=Alu.is_equal, op1=Alu.mult)
                                    sctr[0] += 1
                                    sv = svh[:]
                                first = done == 0
                                if 'nomm' in ABL:
                                    done += 1
                                    continue
                                if swapT:
                                    nc.tensor.matmul(ps[:], lhsT=msl,
                                                     rhs=sv,
                                                     start=first,
                                                     stop=(done == tot - 1))
                                else:
                                    nc.tensor.matmul(ps[:], lhsT=sv,
                                                     rhs=msl,
                                                     start=first,
                                                     stop=(done == tot - 1))
                                done += 1
                            pending.append((b, ps))
                            flush(DEFER * SBW)
                    flush(0)

                def fin1(b, ps):
                    # ps = Ax_b [dest, D] fp32 PSUM
                    if 'nofin' in ABL:
                        return
                    rows = min(P, NSH - b * P)
                    t1 = smp.tile([P, D], f32, tag="t1")
                    if 'nomm' in ABL or 'noseg' in ABL:
                        nc.vector.memset(t1[:], 1.0)
                    elif ACTMIN:
                        # Ax + 1, PSUM -> SBUF
                        nc.vector.tensor_scalar_add(t1[:rows, :],
                                                    ps[:rows, :], 1.0)
                    else:
                        nc.scalar.activation(t1[:rows, :], ps[:rows, :],
                                             Act.Identity, bias=1.0)
                    if not fused:
                        nc.vector.tensor_copy(ax_all[:, b * P:(b + 1) * P],
                                              ps[:])
                    h2 = smp.tile([P, D], bf16, tag="h2")
                    # H' = x * (1 + Ax)  (bf16 out)
                    nc.vector.tensor_tensor(h2[:rows, :], t1[:rows, :],
                                            xall[:rows, b, :], op=Alu.mult)
                    odma.dma_start(h2sh[b * P:b * P + rows, :],
                                   h2[:rows, :])

                spmm_pass(xtab[:SPLIT, :], xtab[SPLIT:, :], fin1, swapT=False)

                if 'noag' not in ABL:
                    nc.gpsimd.collective_compute(
                        "AllGather", Alu.bypass,
                        replica_groups=[list(range(NCORES))],
                        ins=[h2sh[:]], outs=[h2full[:]])

                def fin2(b, ps):
                    # ps = (Ax+Axx)_b.T [D, dest] fp32 PSUM
                    if 'nofin' in ABL:
                        return
                    rows = min(P, NSH - b * P)
                    k = b % NROT
                    sT = smp.tile([P, P], bf16, tag="sT")
                    if 'nomm' in ABL or 'noseg' in ABL:
                        nc.vector.memset(sT[:], 0.0)
                    elif ACTMIN:
                        nc.vector.tensor_copy(sT[:], ps[:])
                    else:
                        nc.scalar.activation(sT[:], ps[:], Act.Copy)
                    pp = ppre.tile([P, P], f32, tag="pre")
                    nc.tensor.matmul(pp[:], lhsT=onesr[k][:], rhs=bsr[k][:],
                                     start=True, stop=False)
                    if fused:
                        nc.tensor.matmul(pp[:], lhsT=sT[:], rhs=wr[k][:],
                                         start=False, stop=True)
                    else:
                        nc.tensor.matmul(pp[:], lhsT=sT[:], rhs=wr[k][:],
                                         start=False, stop=False)
                        tp = ppre.tile([P, P], f32, tag="tp")
                        nc.tensor.transpose(tp[:],
                                            ax_all[:, b * P:(b + 1) * P],
                                            idr[k][:])
                        axT = smp.tile([P, P], bf16, tag="axT")
                        nc.vector.tensor_copy(axT[:], tp[:])
                        nc.tensor.matmul(pp[:], lhsT=axT[:], rhs=wdr[k][:],
                                         start=False, stop=True)
                    pre_sb = smp.tile([P, P], f32, tag="presb")
                    if ACTMIN:
                        nc.vector.tensor_copy(pre_sb[:], pp[:])
                    else:
                        nc.scalar.activation(pre_sb[:], pp[:], Act.Copy)
                    nc.sync.dma_start(pre_o[b * P:b * P + rows, :],
                                      pre_sb[:rows, :])
                    ex = smp.tile([P, P], f32, tag="ex")
                    nc.scalar.activation(ex[:], pp[:], Act.Exp)
                    # elu(x) = max(x, min(exp(x),1) - 1)
                    t = smp.tile([P, P], f32, tag="t")
                    nc.vector.tensor_scalar(t[:], ex[:], 1.0, -1.0,
                                            op0=Alu.min, op1=Alu.add)
                    elu = smp.tile([P, P], f32, tag="elu")
                    nc.vector.tensor_tensor(elu[:], t[:], pre_sb[:],
                                            op=Alu.max)
                    odma.dma_start(elu_o[b * P:b * P + rows, :],
                                   elu[:rows, :])

                if 'p2fromx2' in ABL:
                    spmm_pass(xtab[:SPLIT, :], xtab[SPLIT:, :], fin2,
                              swapT=True)
                else:
                    spmm_pass(h2full[:SPLIT, :], h2full[SPLIT:, :], fin2,
                              swapT=True)

            for _ in range(reps):
                run_once()

    nc.compile()
    return nc


def _get_program(pp, reps=1, fused=True):
    key = (pp["caps_lo"], pp["caps_hi"], reps, fused,
           SBW, TSDVE, ACTMIN, ODMA, SPKT, MBUF, SPOOL, PSEG, DEFER, HOSTS,
           os.environ.get("ABL", ""))
    if key not in _cache:
        _cache[key] = _build(list(pp["caps_lo"]), list(pp["caps_hi"]),
                             pp["TC"], pp["CL"], pp["CH"], reps=reps,
                             fused=fused)
    return _cache[key]


def _in_maps(pp, features, W1, b1, W2, b2, fused=True):
    feats = np.ascontiguousarray(np.asarray(features, np.float32))
    xtab = np.ascontiguousarray(feats.astype(BF16))
    w1 = np.asarray(W1, np.float32)
    w2 = np.asarray(W2, np.float32)
    bsum = (np.asarray(b1, np.float32) + np.asarray(b2, np.float32))[None, :]
    maps = []
    for c in range(NCORES):
        m = {
            "xtab": xtab,
            "xshard": feats[c * NSH:(c + 1) * NSH],
            "idxlo": pp["idxlo"][c],
            "idxhi": pp["idxhi"][c],
            "rowloc": pp["rowloc"][c],
            "vhi": pp["vhi"][c],
            "sall": pp["sall"][c],
            "wsum": np.ascontiguousarray(w2.T) if not fused
                    else np.ascontiguousarray(w1.T),
            "bsum": bsum,
        }
        if not fused:
            m["wdiff"] = np.ascontiguousarray((w1 - w2).T)
        maps.append(m)
    return maps


def kernel(features, adj_row, adj_col, adj_val, W1, b1, W2, b2):
    from concourse.bass_utils import run_bass_kernel_spmd

    fused = np.array_equal(np.asarray(W1), np.asarray(W2))
    pp = _preprocess(adj_row, adj_col, adj_val)
    nc = _get_program(pp, fused=fused)
    maps = _in_maps(pp, features, W1, b1, W2, b2, fused=fused)
    res = run_bass_kernel_spmd(nc, maps, list(range(NCORES)))
    pre = np.concatenate([res.results[c]["pre"] for c in range(NCORES)], axis=0)
    out = np.concatenate([res.results[c]["eluout"] for c in range(NCORES)], axis=0)
    return (pre, out)


# revision 30
# speedup vs baseline: 2.1211x; 2.1211x over previous
"""GNN message-passing layer (GSS GNNLayer) on 8 Trainium2 NeuronCores.

Math (see reference):
    Ax   = A @ x                 (sparse COO, E edges)
    pre1 = Ax @ W1.T + b1
    Axx  = A @ (Ax * x)
    pre2 = Axx @ W2.T + b2
    pre  = pre1 + pre2 ; out = elu(pre) ; return (pre, out)

Distribution: row-partition by destination node; core c owns dest rows
[c*5000, (c+1)*5000). Edges are bucketed by (core, dest-block of 128,
lo/hi source-table half) on the host, sorted by source within a bucket
(HBM locality), and padded to chunks of 128 with val=0 edges.

All data-plane tensors are bf16 (2e-2 rel-err budget; bf16 lands ~2e-3).
Pass 2 exploits Ax + Axx = A @ (x + Ax*x): it gathers H' = x*(1+Ax) and
accumulates (Ax+Axx) directly, so with W1 == W2 (always true for this
module's init) a single dense matmul per block finishes the layer:
    pre = (Ax+Axx) @ W1.T + (b1+b2)
(W1 != W2 falls back to an extra Ax @ (W1-W2).T matmul per block.)

SpMM: per chunk of 128 edges the device dma_gather's the 128 source
rows (256 B/row bf16, int16 indices, table halved at row 32768, 4
SWDGE queues round-robin) and accumulates S.T @ M (pass 1) or M.T @ S
(pass 2, transposed result feeds the dense matmul without a PE
transpose) into the block's fp32 PSUM tile, where the bf16 selection
matrix
    S[e,d] = val[e] * (d == rowlocal[e])
is HOST-precomputed and streamed one superblock per DMA (~1 MB each):
sequential HBM reads are far cheaper than ~1400 in-order DVE
tensor_scalar dispatches (-148 us/iter measured). K_HOSTS=0 restores
the on-device build (fused is_equal*val tensor_scalar from an iota
tile; never GpSimd - its tensor ops are ~10x slow on real HW).

Finals: bias via a rank-1 matmul (ones.T @ bsum_row) into the same
PSUM, ELU = max(x, min(exp(x),1)-1) split across Activation + DVE.

SPMD: one program for all 8 cores; per-(block,table) chunk counts are
the max over cores, computed from the actual input, so the program
structure is uniform and only the data differs.
"""

import os
import numpy as np
import ml_dtypes

BF16 = ml_dtypes.bfloat16

N = 40000
D = 128
E = 640000
NCORES = 8
NSH = N // NCORES          # 5000 dest rows per core
P = 128
NB = (NSH + P - 1) // P    # 40 dest blocks per core (last has 8 rows)
SPLIT = 32768              # int16 gather index limit
NQ = 4                     # SWDGE queues for gathers

# tuning knobs (baked into the compiled program; cache-keyed)
SBW = int(os.environ.get("K_SBW", "2"))      # blocks per gather super-block
TSDVE = int(os.environ.get("K_TSDVE", "3"))  # S-builds on DVE out of 3
ACTMIN = os.environ.get("K_ACTMIN", "1") == "1"  # Act engine: Exp only
ODMA = os.environ.get("K_ODMA", "sync")      # output-DMA engine
SPKT = os.environ.get("K_SPKT", "0") == "1"  # dma_gather single_packet
MBUF = int(os.environ.get("K_MBUF", "3"))    # gather-tile ring depth
SPOOL = int(os.environ.get("K_SPOOL", "12"))  # S-tile ring depth
PSEG = int(os.environ.get("K_PSEG", "4"))    # segment-PSUM ring depth
DEFER = int(os.environ.get("K_DEFER", "0"))  # finalize deferral (superblocks)
HOSTS = os.environ.get("K_HOSTS", "1") == "1"  # host-built S tiles via DMA

_cache = {}


def _preprocess(adj_row, adj_col, adj_val):
    """Bucket/pad edges; build per-core gather-index and S-descriptor arrays."""
    row = np.asarray(adj_row, np.int64)
    col = np.asarray(adj_col, np.int64)
    val = np.asarray(adj_val, np.float32)

    core = row // NSH
    loc = row - core * NSH
    blk = loc // P
    dloc = (loc % P).astype(np.float32)
    hi = (col >= SPLIT).astype(np.int64)

    key = (core * NB + blk) * 2 + hi          # 0 .. NCORES*NB*2-1
    nkey = NCORES * NB * 2
    order = np.lexsort((col, key))            # bucket-major, source-sorted
    sk = key[order]
    counts = np.bincount(key, minlength=nkey)
    gstart = np.concatenate([[0], np.cumsum(counts)[:-1]])
    pos = np.arange(len(sk)) - gstart[sk]     # rank within its bucket
    cnt = counts.reshape(NCORES, NB, 2)
    caps = np.ceil(cnt / P).astype(np.int64).max(axis=0)   # [NB, 2]
    caps[:, 0] = np.maximum(caps[:, 0], 1)    # every block needs >=1 chunk
    caps_lo = caps[:, 0]
    caps_hi = caps[:, 1]

    # chunk-column layout: per block, lo chunks then hi chunks
    col0 = np.zeros((NB, 2), np.int64)
    run = 0
    for b in range(NB):
        col0[b, 0] = run
        run += caps_lo[b]
        col0[b, 1] = run
        run += caps_hi[b]
    TC = int(run)
    lostart = np.concatenate([[0], np.cumsum(caps_lo)])
    histart = np.concatenate([[0], np.cumsum(caps_hi)])
    CL = int(lostart[-1]) * 8                 # idx cols (16 idx/col)
    CH = max(int(histart[-1]) * 8, 1)

    rowloc = np.zeros((NCORES, P, TC), np.float32)
    vhi = np.zeros((NCORES, P, TC), np.float32)
    idxlo = np.zeros((NCORES, P, CL), np.int16)
    idxhi = np.zeros((NCORES, P, CH), np.int16)
    sall = np.zeros((NCORES, P, TC, P), BF16)   # host-built S tiles

    cS = sk // (NB * 2)
    bS = (sk // 2) % NB
    tS = sk % 2
    dS = dloc[order]
    vS = val[order]
    colS = col[order]

    ccol = col0[bS, tS] + pos // P
    pp = pos % P
    rowloc[cS, pp, ccol] = dS
    vhi[cS, pp, ccol] = vS
    sall[cS, pp, ccol, dS.astype(np.int64)] = vS.astype(BF16)

    reps = 16 * np.arange(8)[None, :]
    m = tS == 0
    q = lostart[bS[m]] * P + pos[m]
    idxlo[cS[m][:, None], (q % 16)[:, None] + reps, (q // 16)[:, None]] = \
        colS[m].astype(np.int16)[:, None]
    m = tS == 1
    if m.any():
        q = histart[bS[m]] * P + pos[m]
        idxhi[cS[m][:, None], (q % 16)[:, None] + reps, (q // 16)[:, None]] = \
            (colS[m] - SPLIT).astype(np.int16)[:, None]

    return dict(caps_lo=tuple(int(x) for x in caps_lo),
                caps_hi=tuple(int(x) for x in caps_hi),
                TC=TC, CL=CL, CH=CH,
                rowloc=rowloc, vhi=vhi,
                idxlo=idxlo, idxhi=idxhi,
                sall=sall.reshape(NCORES, P, TC * P))


def _build(caps_lo, caps_hi, TC, CL, CH, reps=1, fused=True):
    ABL = set(os.environ.get('ABL', '').split(','))
    import concourse.bacc as bacc
    import concourse.mybir as mybir
    import concourse.tile as tile

    f32 = mybir.dt.float32
    bf16 = mybir.dt.bfloat16
    i16 = mybir.dt.int16
    Alu = mybir.AluOpType
    Act = mybir.ActivationFunctionType

    lostart = np.concatenate([[0], np.cumsum(caps_lo)]).astype(int)
    histart = np.concatenate([[0], np.cumsum(caps_hi)]).astype(int)
    col0 = np.zeros((NB, 2), np.int64)
    run = 0
    for b in range(NB):
        col0[b, 0] = run
        run += caps_lo[b]
        col0[b, 1] = run
        run += caps_hi[b]

    nc = bacc.Bacc(None, target_bir_lowering=False, num_swdge_queues=NQ)
    xtab = nc.declare_dram_parameter("xtab", [N, D], bf16, isOutput=False)
    xsh = nc.declare_dram_parameter("xshard", [NSH, D], f32, isOutput=False)
    idxlo_d = nc.declare_dram_parameter("idxlo", [P, CL], i16, isOutput=False)
    idxhi_d = nc.declare_dram_parameter("idxhi", [P, CH], i16, isOutput=False)
    if HOSTS:
        sall_d = nc.declare_dram_parameter("sall", [P, TC * P], bf16,
                                           isOutput=False)
    else:
        rowloc_d = nc.declare_dram_parameter("rowloc", [P, TC], f32,
                                             isOutput=False)
        vhi_d = nc.declare_dram_parameter("vhi", [P, TC], f32, isOutput=False)
    wsum_d = nc.declare_dram_parameter("wsum", [D, D], f32, isOutput=False)
    bsum_d = nc.declare_dram_parameter("bsum", [1, D], f32, isOutput=False)
    if not fused:
        wdiff_d = nc.declare_dram_parameter("wdiff", [D, D], f32, isOutput=False)
    pre_o = nc.declare_dram_parameter("pre", [NSH, D], f32, isOutput=True)
    elu_o = nc.declare_dram_parameter("eluout", [NSH, D], f32, isOutput=True)
    h2sh = nc.dram_tensor("H2_shard", [NSH, D], bf16)
    h2full = nc.dram_tensor("H2_full", [N, D], bf16, addr_space="Shared")

    NSB = (NB + SBW - 1) // SBW
    nlo_sb = [sum(caps_lo[s * SBW:(s + 1) * SBW]) for s in range(NSB)]
    nhi_sb = [sum(caps_hi[s * SBW:(s + 1) * SBW]) for s in range(NSB)]
    NROT = 8

    with tile.TileContext(nc) as tc:
        with (
            tc.tile_pool(name="const", bufs=1) as cpool,
            tc.tile_pool(name="mlo", bufs=MBUF) as mlop,
            tc.tile_pool(name="mhi", bufs=MBUF) as mhip,
            tc.tile_pool(name="sel", bufs=SPOOL) as spool,
            tc.tile_pool(name="ssb", bufs=MBUF) as sspool,
            tc.tile_pool(name="small", bufs=3) as smp,
            tc.tile_pool(name="psum", bufs=PSEG, space="PSUM") as pseg,
            tc.tile_pool(name="psum3", bufs=2, space="PSUM") as ppre,
        ):
            if not HOSTS:
                iota_v = cpool.tile([P, P], bf16)
                nc.gpsimd.iota(iota_v[:], pattern=[[1, P]], base=0,
                               channel_multiplier=0,
                               allow_small_or_imprecise_dtypes=True)
                iota_p = cpool.tile([P, P], bf16)
                nc.vector.tensor_copy(iota_p[:], iota_v[:])
            wsum_t = cpool.tile([D, D], f32)
            nc.sync.dma_start(wsum_t[:], wsum_d[:])
            bsum_t = cpool.tile([1, D], f32)
            nc.sync.dma_start(bsum_t[:], bsum_d[:])
            idxlo_t = cpool.tile([P, CL], i16)
            nc.sync.dma_start(idxlo_t[:], idxlo_d[:])
            idxhi_t = cpool.tile([P, CH], i16)
            nc.sync.dma_start(idxhi_t[:], idxhi_d[:])
            if not HOSTS:
                rowloc_t = cpool.tile([P, TC], f32)
                nc.sync.dma_start(rowloc_t[:], rowloc_d[:])
                vhi_t = cpool.tile([P, TC], f32)
                nc.sync.dma_start(vhi_t[:], vhi_d[:])
            # full x shard resident in SBUF: [dest_p, block, D]
            xall = cpool.tile([P, NB, D], f32)
            for b in range(NB):
                rows = min(P, NSH - b * P)
                nc.sync.dma_start(xall[:rows, b, :],
                                  xsh[b * P:b * P + rows, :])
            # rotating operand copies for the per-block dense finals
            wr, onesr, bsr = [], [], []
            for k in range(NROT):
                t1 = cpool.tile([D, D], bf16, tag=f"wr{k}")
                nc.vector.tensor_copy(t1[:], wsum_t[:])
                wr.append(t1)
                t2 = cpool.tile([1, P], bf16, tag=f"ones{k}")
                nc.vector.memset(t2[:], 1.0)
                onesr.append(t2)
                t3 = cpool.tile([1, D], bf16, tag=f"bsr{k}")
                nc.vector.tensor_copy(t3[:], bsum_t[:])
                bsr.append(t3)
            if not fused:
                from concourse.masks import make_identity
                ident = cpool.tile([P, P], f32)
                make_identity(nc, ident[:])
                wdiff_t = cpool.tile([D, D], f32)
                nc.sync.dma_start(wdiff_t[:], wdiff_d[:])
                wdr, idr = [], []
                for k in range(NROT):
                    t4 = cpool.tile([D, D], bf16, tag=f"wd{k}")
                    nc.vector.tensor_copy(t4[:], wdiff_t[:])
                    wdr.append(t4)
                    t5 = cpool.tile([P, P], f32, tag=f"idr{k}")
                    nc.vector.tensor_copy(t5[:], ident[:])
                    idr.append(t5)
                ax_all = cpool.tile([P, NB * P], f32)

            odma = nc.sync if ODMA == "sync" else nc.scalar
            qctr = [0]
            sctr = [0]

            def run_once():
                def spmm_pass(tlo, thi, finalize, swapT):
                    # Deferring finalize emission by DEFER superblocks keeps
                    # the next superblock's S-builds ahead of fin ops in the
                    # in-order engine queues (software pipelining).
                    pending = []

                    def flush(limit):
                        while len(pending) > limit:
                            fb, fps = pending.pop(0)
                            finalize(fb, fps)

                    for s in range(NSB):
                        b0 = s * SBW
                        nlo, nhi = nlo_sb[s], nhi_sb[s]
                        if HOSTS:
                            c0 = int(col0[b0, 0])
                            nsb = nlo + nhi
                            s_sb = sspool.tile([P, nsb, P], bf16, tag="ssb")
                            nc.sync.dma_start(
                                s_sb[:],
                                sall_d[:, c0 * P:(c0 + nsb) * P])
                        mlo = mlop.tile([P, nlo, D], bf16, tag="mlo")
                        if 'nogather' in ABL:
                            nc.sync.dma_start(mlo[:, 0, :], tlo[0:P, :])
                        else:
                            h1 = nlo // 2
                            for (g0, g1) in ((0, h1), (h1, nlo)):
                                if g1 <= g0:
                                    continue
                                nc.gpsimd.dma_gather(
                                    out_ap=mlo[:, g0:g1, :], in_ap=tlo,
                                    idxs_ap=idxlo_t[:, (lostart[b0] + g0) * 8:(lostart[b0] + g1) * 8],
                                    num_idxs=(g1 - g0) * P,
                                    num_idxs_reg=(g1 - g0) * P,
                                    elem_size=D, single_packet=SPKT,
                                    queue_num=qctr[0] % NQ)
                                qctr[0] += 1
                        if nhi:
                            mhi = mhip.tile([P, nhi, D], bf16, tag="mhi")
                            if 'nogather' in ABL:
                                nc.sync.dma_start(mhi[:, 0, :], tlo[0:P, :])
                            else:
                                nc.gpsimd.dma_gather(
                                    out_ap=mhi[:], in_ap=thi,
                                    idxs_ap=idxhi_t[:, histart[b0] * 8:(histart[b0] + nhi) * 8],
                                    num_idxs=nhi * P, num_idxs_reg=nhi * P,
                                    elem_size=D, single_packet=SPKT,
                                    queue_num=qctr[0] % NQ)
                                qctr[0] += 1
                        for i in range(min(SBW, NB - b0)):
                            b = b0 + i
                            ps = pseg.tile([P, P], f32, tag="seg")
                            tot = caps_lo[b] + caps_hi[b]
                            done = 0
                            glo = sum(caps_lo[b0:b])
                            ghi = sum(caps_hi[b0:b])
                            for j in range(tot):
                                if j < caps_lo[b]:
                                    c = int(col0[b, 0]) + j
                                    msl = mlo[:, glo + j, :]
                                else:
                                    c = int(col0[b, 1]) + (j - caps_lo[b])
                                    msl = mhi[:, ghi + (j - caps_lo[b]), :]
                                if 'noseg' in ABL:
                                    done += 1
                                    continue
                                if HOSTS:
                                    sv = s_sb[:, c - c0, :]
                                else:
                                    svh = spool.tile([P, P], bf16, tag="S")
                                    # DVE : GpSimd split of the S-builds
                                    if sctr[0] % 3 < TSDVE:
                                        nc.vector.tensor_scalar(
                                            svh[:], iota_v[:],
                                            rowloc_t[:, c:c + 1],
                                            vhi_t[:, c:c + 1],
                                            op0=Alu.is_equal, op1=Alu.mult)
                                    else:
                                        nc.gpsimd.tensor_scalar(
                                            svh[:], iota_p[:],
                                            rowloc_t[:, c:c + 1],
                                            vhi_t[:, c:c + 1],
                                            op0=Alu.is_equal, op1=Alu.mult)
                                    sctr[0] += 1
                                    sv = svh[:]
                                first = done == 0
                                if 'nomm' in ABL:
                                    done += 1
                                    continue
                                if swapT:
                                    nc.tensor.matmul(ps[:], lhsT=msl,
                                                     rhs=sv,
                                                     start=first,
                                                     stop=(done == tot - 1))
                                else:
                                    nc.tensor.matmul(ps[:], lhsT=sv,
                                                     rhs=msl,
                                                     start=first,
                                                     stop=(done == tot - 1))
                                done += 1
                            pending.append((b, ps))
                            flush(DEFER * SBW)
                    flush(0)

                def fin1(b, ps):
                    # ps = Ax_b [dest, D] fp32 PSUM
                    if 'nofin' in ABL:
                        return
                    rows = min(P, NSH - b * P)
                    t1 = smp.tile([P, D], f32, tag="t1")
                    if 'nomm' in ABL or 'noseg' in ABL:
                        nc.vector.memset(t1[:], 1.0)
                    elif ACTMIN:
                        # Ax + 1, PSUM -> SBUF
                        nc.vector.tensor_scalar_add(t1[:rows, :],
                                                    ps[:rows, :], 1.0)
                    else:
                        nc.scalar.activation(t1[:rows, :], ps[:rows, :],
                                             Act.Identity, bias=1.0)
                    if not fused:
                        nc.vector.tensor_copy(ax_all[:, b * P:(b + 1) * P],
                                              ps[:])
                    h2 = smp.tile([P, D], bf16, tag="h2")
                    # H' = x * (1 + Ax)  (bf16 out)
                    nc.vector.tensor_tensor(h2[:rows, :], t1[:rows, :],
                                            xall[:rows, b, :], op=Alu.mult)
                    odma.dma_start(h2sh[b * P:b * P + rows, :],
                                   h2[:rows, :])

                spmm_pass(xtab[:SPLIT, :], xtab[SPLIT:, :], fin1, swapT=False)

                if 'noag' not in ABL:
                    nc.gpsimd.collective_compute(
                        "AllGather", Alu.bypass,
                        replica_groups=[list(range(NCORES))],
                        ins=[h2sh[:]], outs=[h2full[:]])

                def fin2(b, ps):
                    # ps = (Ax+Axx)_b.T [D, dest] fp32 PSUM
                    if 'nofin' in ABL:
                        return
                    rows = min(P, NSH - b * P)
                    k = b % NROT
                    sT = smp.tile([P, P], bf16, tag="sT")
                    if 'nomm' in ABL or 'noseg' in ABL:
                        nc.vector.memset(sT[:], 0.0)
                    elif ACTMIN:
                        nc.vector.tensor_copy(sT[:], ps[:])
                    else:
                        nc.scalar.activation(sT[:], ps[:], Act.Copy)
                    pp = ppre.tile([P, P], f32, tag="pre")
                    nc.tensor.matmul(pp[:], lhsT=onesr[k][:], rhs=bsr[k][:],
                                     start=True, stop=False)
                    if fused:
                        nc.tensor.matmul(pp[:], lhsT=sT[:], rhs=wr[k][:],
                                         start=False, stop=True)
                    else:
                        nc.tensor.matmul(pp[:], lhsT=sT[:], rhs=wr[k][:],
                                         start=False, stop=False)
                        tp = ppre.tile([P, P], f32, tag="tp")
                        nc.tensor.transpose(tp[:],
                                            ax_all[:, b * P:(b + 1) * P],
                                            idr[k][:])
                        axT = smp.tile([P, P], bf16, tag="axT")
                        nc.vector.tensor_copy(axT[:], tp[:])
                        nc.tensor.matmul(pp[:], lhsT=axT[:], rhs=wdr[k][:],
                                         start=False, stop=True)
                    pre_sb = smp.tile([P, P], f32, tag="presb")
                    if ACTMIN:
                        nc.vector.tensor_copy(pre_sb[:], pp[:])
                    else:
                        nc.scalar.activation(pre_sb[:], pp[:], Act.Copy)
                    nc.sync.dma_start(pre_o[b * P:b * P + rows, :],
                                      pre_sb[:rows, :])
                    ex = smp.tile([P, P], f32, tag="ex")
                    nc.scalar.activation(ex[:], pp[:], Act.Exp)
                    # elu(x) = max(x, min(exp(x),1) - 1)
                    t = smp.tile([P, P], f32, tag="t")
                    nc.vector.tensor_scalar(t[:], ex[:], 1.0, -1.0,
                                            op0=Alu.min, op1=Alu.add)
                    elu = smp.tile([P, P], f32, tag="elu")
                    nc.vector.tensor_tensor(elu[:], t[:], pre_sb[:],
                                            op=Alu.max)
                    odma.dma_start(elu_o[b * P:b * P + rows, :],
                                   elu[:rows, :])

                if 'p2fromx2' in ABL:
                    spmm_pass(xtab[:SPLIT, :], xtab[SPLIT:, :], fin2,
                              swapT=True)
                else:
                    spmm_pass(h2full[:SPLIT, :], h2full[SPLIT:, :], fin2,
                              swapT=True)

            for _ in range(reps):
                run_once()

    nc.compile()
    return nc


def _get_program(pp, reps=1, fused=True):
    key = (pp["caps_lo"], pp["caps_hi"], reps, fused,
           SBW, TSDVE, ACTMIN, ODMA, SPKT, MBUF, SPOOL, PSEG, DEFER, HOSTS,
           os.environ.get("ABL", ""))
    if key not in _cache:
        _cache[key] = _build(list(pp["caps_lo"]), list(pp["caps_hi"]),
                             pp["TC"], pp["CL"], pp["CH"], reps=reps,
                             fused=fused)
    return _cache[key]


def _in_maps(pp, features, W1, b1, W2, b2, fused=True):
    feats = np.ascontiguousarray(np.asarray(features, np.float32))
    xtab = np.ascontiguousarray(feats.astype(BF16))
    w1 = np.asarray(W1, np.float32)
    w2 = np.asarray(W2, np.float32)
    bsum = (np.asarray(b1, np.float32) + np.asarray(b2, np.float32))[None, :]
    maps = []
    for c in range(NCORES):
        m = {
            "xtab": xtab,
            "xshard": feats[c * NSH:(c + 1) * NSH],
            "idxlo": pp["idxlo"][c],
            "idxhi": pp["idxhi"][c],
            "rowloc": pp["rowloc"][c],
            "vhi": pp["vhi"][c],
            "sall": pp["sall"][c],
            "wsum": np.ascontiguousarray(w2.T) if not fused
                    else np.ascontiguousarray(w1.T),
            "bsum": bsum,
        }
        if not fused:
            m["wdiff"] = np.ascontiguousarray((w1 - w2).T)
        maps.append(m)
    return maps


def kernel(features, adj_row, adj_col, adj_val, W1, b1, W2, b2):
    from concourse.bass_utils import run_bass_kernel_spmd

    fused = np.array_equal(np.asarray(W1), np.asarray(W2))
    pp = _preprocess(adj_row, adj_col, adj_val)
    nc = _get_program(pp, fused=fused)
    maps = _in_maps(pp, features, W1, b1, W2, b2, fused=fused)
    res = run_bass_kernel_spmd(nc, maps, list(range(NCORES)))
    pre = np.concatenate([res.results[c]["pre"] for c in range(NCORES)], axis=0)
    out = np.concatenate([res.results[c]["eluout"] for c in range(NCORES)], axis=0)
    return (pre, out)
